# revision 3
# baseline (speedup 1.0000x reference)
"""Performer (FAVOR+ causal linear attention) encoder on 8 NeuronCores.

Sharding: DP over batch (2 groups of 4 cores) x TP-4 within group
(heads 4-way for attention/QKV/Wo, hidden 4-way for FF). Two AllReduces
per layer over each 4-core group (attention-out partial, FF partial).

GEMMs run bf16xbf16 with fp32 accumulation (weights + transposed
activations cast to bf16); attention feature/scan math stays fp32
(f32r for wide matmuls). Rotary uses a host-side even/odd column
permutation of Wq/Wk so rotate-every-two becomes contiguous 32-row ops.
Attention: chunked prefix scan, C=256. Query features exploit q-side
scale invariance (per-row max/diag cancel in num/denom); key features
are exact, with the k-side eps term restored via a rank-1 correction
(beta * rowsum(q') * mask-matmul cumsum of V).
"""

import numpy as np

B, N, D, H, DH, M, FF, DEPTH = 2, 2048, 1024, 16, 64, 266, 4096, 6
TP = 4
HPC = H // TP            # 4 heads per core
HW = HPC * DH            # 256
FFS = FF // TP           # 1024
C = 256                  # attention chunk
NCH = N // C
NT = N // 128
DT = D // 128
MT = [(0, 128), (128, 128), (256, 10)]
DN = float(DH) ** -0.25
RATIO = float(M) ** -0.5
EPS_F = 1e-4
EPS_C = 1e-6
BETA = RATIO * EPS_F

_CACHE = {}
TRACE = False          # opt-in: capture an NTFF/perfetto trace on next call
LAST_RESULT = None     # BassKernelResults of the most recent run


def _build():
    import concourse.bass as bass
    import concourse.tile as tile
    from concourse import mybir, bacc
    from concourse.masks import make_identity

    f32 = mybir.dt.float32
    bf16 = mybir.dt.bfloat16
    f32r = mybir.dt.float32r

    def r(ap):
        return ap.bitcast(f32r)

    nc = bacc.Bacc("TRN2", target_bir_lowering=False, debug=False, num_devices=8)

    def inp(name, shape, dt=f32):
        return nc.declare_dram_parameter(name, list(shape), dt, isOutput=False)

    x_in = inp("x", [N, D])
    cosT = inp("cosT", [32, N], f32r)
    sinT = inp("sinT", [32, N], f32r)
    mask_in = inp("mask", [2, 128, C])
    wq_in = inp("wq", [DEPTH, D, HW], bf16)
    wk_in = inp("wk", [DEPTH, D, HW], bf16)
    wv_in = inp("wv", [DEPTH, D, HW], bf16)
    bq_in = inp("bq", [DEPTH, HW])
    bk_in = inp("bk", [DEPTH, HW])
    bv_in = inp("bv", [DEPTH, HW])
    wo_in = inp("wo", [DEPTH, HW, D], bf16)
    w1_in = inp("w1", [DEPTH, D, FFS], bf16)
    b1_in = inp("b1h", [DEPTH, FFS])
    w2_in = inp("w2", [DEPTH, FFS, D], bf16)
    b2_in = inp("b2q", [DEPTH, D])
    pj_in = inp("projT", [DEPTH, DH, M], f32r)
    out_p = nc.declare_dram_parameter("out", [N, D], f32, isOutput=True)

    x_res = nc.dram_tensor("x_res", [N, D], f32)
    cc_in = [nc.dram_tensor(f"cc_in{i}", [N, D], f32) for i in range(2 * DEPTH)]
    cc_out = [nc.dram_tensor(f"cc_out{i}", [N, D], f32) for i in range(2 * DEPTH)]
    mu_dr = [nc.dram_tensor(f"mu{i}", [1, 1], f32) for i in range(DEPTH * HPC)]
    groups = [[0, 1, 2, 3], [4, 5, 6, 7]]

    Exp = mybir.ActivationFunctionType.Exp
    Gelu = mybir.ActivationFunctionType.Gelu_apprx_tanh
    Sqrt = mybir.ActivationFunctionType.Sqrt
    Square = mybir.ActivationFunctionType.Square
    Add = mybir.AluOpType.add
    Sub = mybir.AluOpType.subtract
    Mult = mybir.AluOpType.mult

    def bcast_row(dst, src_ap):
        # dst [128, F] <- broadcast of a [F] dram row across partitions
        nc.sync.dma_start(out=dst, in_=bass.AP(
            tensor=src_ap.tensor, offset=src_ap.offset,
            ap=[[0, 128]] + list(src_ap.ap)))

    with tile.TileContext(nc) as tc:
        const = tc.alloc_tile_pool(name="const", bufs=1)
        big = tc.alloc_tile_pool(name="big", bufs=1)
        wpool = tc.alloc_tile_pool(name="w", bufs=1)
        sm = tc.alloc_tile_pool(name="sm", bufs=2)
        sm3 = tc.alloc_tile_pool(name="sm3", bufs=3)
        smA = tc.alloc_tile_pool(name="smA", bufs=1)
        ev = tc.alloc_tile_pool(name="ev", bufs=2)
        hp = tc.alloc_tile_pool(name="hp", bufs=1)
        pT = tc.alloc_tile_pool(name="pT", bufs=2, space="PSUM")
        pB = tc.alloc_tile_pool(name="pB", bufs=3, space="PSUM")
        pS = tc.alloc_tile_pool(name="pS", bufs=3, space="PSUM")

        ident = const.tile([128, 128], f32)
        make_identity(nc, ident)
        ident_bf = const.tile([128, 128], bf16)
        make_identity(nc, ident_bf)
        cos_sb = const.tile([32, N], f32r)
        sin_sb = const.tile([32, N], f32r)
        nc.sync.dma_start(out=cos_sb, in_=cosT.ap())
        nc.sync.dma_start(out=sin_sb, in_=sinT.ap())
        mask_sb = const.tile([128, 2, C], f32)
        nc.sync.dma_start(out=mask_sb, in_=mask_in.ap().rearrange("s p t -> p s t"))
        ones05 = const.tile([64, 1], f32)
        nc.vector.memset(ones05, 0.5 * DN * DN)
        epsb = const.tile([128, 1], f32)
        nc.vector.memset(epsb, 1e-5)
        lnratio = const.tile([128, 1], f32)
        nc.vector.memset(lnratio, float(np.log(RATIO)))

        nc.sync.dma_start(out=x_res.ap(), in_=x_in.ap())
        xr = x_res.ap().rearrange("(t p) d -> p t d", p=128)

        def layernorm_tile(xt, out_tile):
            st = sm.tile([128, 2, 6], f32, tag="bnst")
            nc.vector.bn_stats(out=st[:, 0], in_=xt[:, 0:512])
            nc.vector.bn_stats(out=st[:, 1], in_=xt[:, 512:1024])
            mv = sm.tile([128, 2], f32, tag="bnmv")
            nc.vector.bn_aggr(out=mv, in_=st)
            rs = sm.tile([128, 1], f32, tag="bnrs")
            nc.scalar.activation(out=rs, in_=mv[:, 1:2], func=Sqrt, bias=epsb)
            nc.vector.reciprocal(out=rs, in_=rs)
            nc.vector.tensor_scalar(out=out_tile, in0=xt, scalar1=mv[:, 0:1],
                                    scalar2=rs, op0=Sub, op1=Mult)

        def transpose128(dst, src):
            ps = pT.tile([128, 128], f32, tag="tps")
            if src.dtype == bf16:
                pp = ps.bitcast(bf16)[: src.shape[-1], : src.shape[0]]
                idn = ident_bf
            else:
                pp = ps[: src.shape[-1], : src.shape[0]]
                idn = ident
            nc.tensor.transpose(pp, src, idn)
            nc.vector.tensor_copy(out=dst, in_=pp)

        def ln_transpose_pass(get_xt, yT_tag="yT"):
            # LN each token tile -> transpose into bf16 [128, DT, N]
            yT = big.tile([128, DT, N], bf16, tag=yT_tag)
            for t in range(NT):
                xt = get_xt(t)
                yt = ev.tile([128, D], f32, tag="yln")
                layernorm_tile(xt, yt)
                for d_ in range(DT):
                    transpose128(yT[:, d_, t * 128:(t + 1) * 128],
                                 yt[:, d_ * 128:(d_ + 1) * 128])
            return yT

        def load_x(t):
            xt = ev.tile([128, D], f32, tag="xtile")
            nc.sync.dma_start(out=xt, in_=xr[:, t])
            return xt

        for layer in range(DEPTH):
            wq_sb = wpool.tile([128, DT, HW], bf16, tag="wq")
            wk_sb = wpool.tile([128, DT, HW], bf16, tag="wk")
            wv_sb = wpool.tile([128, DT, HW], bf16, tag="wv")
            wo_sb = wpool.tile([128, 2, D], bf16, tag="wo")
            pj_sb = wpool.tile([64, M], f32r, tag="pj")
            bq_sb = wpool.tile([64, HPC], f32, tag="bq")
            bk_sb = wpool.tile([64, HPC], f32, tag="bk")
            bv_bc = wpool.tile([128, HW], f32, tag="bvbc")
            nc.sync.dma_start(out=wq_sb, in_=wq_in.ap()[layer].rearrange("(k p) c -> p k c", p=128))
            nc.sync.dma_start(out=wk_sb, in_=wk_in.ap()[layer].rearrange("(k p) c -> p k c", p=128))
            nc.sync.dma_start(out=wv_sb, in_=wv_in.ap()[layer].rearrange("(k p) c -> p k c", p=128))
            nc.sync.dma_start(out=wo_sb, in_=wo_in.ap()[layer].rearrange("(k p) c -> p k c", p=128))
            nc.sync.dma_start(out=pj_sb, in_=pj_in.ap()[layer])
            nc.sync.dma_start(out=bq_sb, in_=bq_in.ap()[layer].rearrange("(h p) -> p h", p=64))
            nc.sync.dma_start(out=bk_sb, in_=bk_in.ap()[layer].rearrange("(h p) -> p h", p=64))
            bcast_row(bv_bc, bv_in.ap()[layer])

            yT = ln_transpose_pass(load_x)

            # v natural [tok, HW] (bf16, feeds V_aug copies)
            v_nat = big.tile([128, NT, HW], bf16, tag="vnat")
            for t in range(NT):
                ps = pB.tile([128, 512], f32, tag="pb")
                for kt in range(DT):
                    nc.tensor.matmul(ps[:, :HW], yT[:, kt, t * 128:(t + 1) * 128],
                                     wv_sb[:, kt], start=(kt == 0), stop=(kt == DT - 1))
                nc.vector.tensor_add(out=v_nat[:, t], in0=ps[:, :HW], in1=bv_bc)

            o_nat = big.tile([128, NT, HW], bf16, tag="onat")

            for h in range(HPC):
                # qh/kh [64, N] fp32, rotary applied
                qh = hp.tile([64, N], f32r, tag="qh")
                kh = hp.tile([64, N], f32r, tag="kh")
                for (buf, wsb, bsb) in ((qh, wq_sb, bq_sb), (kh, wk_sb, bk_sb)):
                    for blk in range(4):
                        ps = pB.tile([128, 512], f32, tag="pb")
                        for kt in range(DT):
                            nc.tensor.matmul(ps[:64], wsb[:, kt, h * 64:(h + 1) * 64],
                                             yT[:, kt, blk * 512:(blk + 1) * 512],
                                             start=(kt == 0), stop=(kt == DT - 1))
                        nc.vector.tensor_scalar_add(out=buf[:, blk * 512:(blk + 1) * 512],
                                                    in0=ps[:64], scalar1=bsb[:, h:h + 1])
                    # rotary in place: rows [0:32]=E, [32:64]=O (host-permuted).
                    # walrus requires equal SBUF base partitions for DVE
                    # inputs, so O rows bounce through base-0 tiles via DMA.
                    for rb_ in range(2):
                        sl = slice(rb_ * 1024, (rb_ + 1) * 1024)
                        qE = buf[0:32, sl]
                        cs, sn = cos_sb[:, sl], sin_sb[:, sl]
                        qOc = sm3.tile([32, 1024], f32r, tag="rot")
                        t1 = sm3.tile([32, 1024], f32r, tag="rot")
                        t2 = sm3.tile([32, 1024], f32r, tag="rot")
                        nc.sync.dma_start(out=qOc, in_=buf[32:64, sl])
                        nc.vector.tensor_mul(out=t1, in0=qE, in1=cs)
                        nc.vector.tensor_mul(out=t2, in0=qOc, in1=sn)
                        nc.vector.tensor_tensor(out=t1, in0=t1, in1=t2, op=Sub)
                        nc.vector.tensor_mul(out=t2, in0=qOc, in1=cs)
                        nc.vector.tensor_mul(out=qOc, in0=qE, in1=sn)
                        nc.vector.tensor_tensor(out=t2, in0=t2, in1=qOc, op=Add)
                        nc.vector.tensor_copy(out=qE, in_=t1)
                        nc.sync.dma_start(out=buf[32:64, sl], in_=t2)

                # delta_k[t] (col layout) and global max mu of ddk
                dcol = hp.tile([128, NT], f32, tag="dcol")
                mx = hp.tile([128, 12], f32, tag="mxt")
                nc.vector.memset(mx, -1e30)
                for t in range(NT):
                    ksq = sm.tile([64, 128], f32, tag="ksq")
                    nc.scalar.activation(out=ksq, in_=kh[:, t * 128:(t + 1) * 128],
                                         func=Square)
                    psd = pS.tile([128, 66], f32, tag="ps")
                    nc.tensor.matmul(psd[:, 0:1], ksq, ones05, start=True, stop=True)
                    nc.vector.tensor_copy(out=dcol[:, t:t + 1], in_=psd[:, 0:1])
                for mi, (m0, mw) in enumerate(MT):
                    for blk in range(4):
                        ps = pB.tile([128, 512], f32, tag="pb")
                        nc.tensor.matmul(ps[:mw], r(pj_sb[:, m0:m0 + mw]),
                                         r(kh[:, blk * 512:(blk + 1) * 512]),
                                         start=True, stop=True)
                        nc.vector.reduce_max(out=mx[:mw, mi * 4 + blk:mi * 4 + blk + 1],
                                             in_=ps[:mw], axis=mybir.AxisListType.X)
                mxr = sm.tile([128, 1], f32, tag="mxr")
                nc.vector.reduce_max(out=mxr, in_=mx, axis=mybir.AxisListType.X)
                mxT = sm.tile([1, 128], f32, tag="mxT")
                transpose128(mxT, mxr)
                mu = sm.tile([1, 1], f32, tag="mu")
                nc.vector.reduce_max(out=mu, in_=mxT, axis=mybir.AxisListType.X)
                md = mu_dr[layer * HPC + h]
                nc.sync.dma_start(out=md.ap(), in_=mu)
                mub = hp.tile([128, 1], f32, tag="mub")
                nc.sync.dma_start(out=mub, in_=bass.AP(tensor=md, offset=0,
                                                       ap=[[0, 128], [1, 1]]))
                nmu = hp.tile([128, 1], f32, tag="nmu")
                nc.vector.tensor_scalar_mul(out=nmu, in0=mub, scalar1=-1.0)
                acol = hp.tile([128, NT], f32, tag="acol")
                nc.scalar.activation(out=acol, in_=dcol, func=Exp, scale=-1.0,
                                     bias=lnratio)

                S_sb = hp.tile([128, 3, 66], f32, tag="Ssb")
                nc.vector.memset(S_sb, 0.0)
                nc.vector.memset(S_sb[:, :, 65:66], 1.0)

                for ci in range(NCH):
                    c0 = ci * C
                    # per-chunk features
                    EkT = smA.tile([128, 3, C], f32r, tag="EkT")
                    Qp = smA.tile([128, 3, C], f32r, tag="Qp")
                    for mi, (m0, mw) in enumerate(MT):
                        ps = pB.tile([128, 512], f32, tag="pb")
                        nc.tensor.matmul(ps[:mw, :C], r(pj_sb[:, m0:m0 + mw]),
                                         r(kh[:, c0:c0 + C]), start=True, stop=True)
                        nc.scalar.activation(out=EkT[:mw, mi], in_=ps[:mw, :C],
                                             func=Exp, bias=nmu[:mw])
                        ps2 = pB.tile([128, 512], f32, tag="pb")
                        nc.tensor.matmul(ps2[:mw, :C], r(pj_sb[:, m0:m0 + mw]),
                                         r(qh[:, c0:c0 + C]), start=True, stop=True)
                        nc.scalar.activation(out=Qp[:mw, mi], in_=ps2[:mw, :C], func=Exp)
                        nc.vector.tensor_scalar_add(out=Qp[:mw, mi], in0=Qp[:mw, mi],
                                                    scalar1=EPS_F)
                    # k' natural per chunk [tok, M]
                    kp = smA.tile([128, 2, M], f32, tag="kp")
                    for st in range(2):
                        ts0 = c0 + st * 128
                        ps = pB.tile([128, 512], f32, tag="pb")
                        nc.tensor.matmul(ps[:, :M], r(kh[:, ts0:ts0 + 128]), r(pj_sb),
                                         start=True, stop=True)
                        nb = sm.tile([128, 1], f32, tag="nbias")
                        tt = ci * 2 + st
                        nc.vector.tensor_scalar(out=nb, in0=dcol[:, tt:tt + 1],
                                                scalar1=mub, scalar2=-1.0,
                                                op0=Add, op1=Mult)
                        nc.scalar.activation(out=kp[:, st], in_=ps[:, :M], func=Exp,
                                             bias=nb)
                        nc.vector.tensor_scalar(out=kp[:, st], in0=kp[:, st],
                                                scalar1=RATIO, scalar2=RATIO * EPS_F,
                                                op0=Mult, op1=Add)
                    # V_aug
                    Va = smA.tile([128, 2, 66], f32, tag="Va")
                    for st in range(2):
                        nc.vector.tensor_copy(out=Va[:, st, 0:64],
                                              in_=v_nat[:, ci * 2 + st, h * 64:(h + 1) * 64])
                        nc.vector.memset(Va[:, st, 64:65], 1.0)
                        nc.vector.memset(Va[:, st, 65:66], 0.0)
                    # G then masked A
                    mA = smA.tile([128, 2, C], f32, tag="mA")
                    for st in range(2):
                        ts0 = c0 + st * 128
                        pg = pB.tile([128, 512], f32, tag="pb")
                        for mi, (m0, mw) in enumerate(MT):
                            nc.tensor.matmul(pg[:, :C], r(EkT[:mw, mi, st * 128:(st + 1) * 128]),
                                             r(Qp[:mw, mi]), start=(mi == 0), stop=(mi == 2))
                        tt = ci * 2 + st
                        nc.vector.tensor_scalar_mul(out=mA[:, st], in0=pg[:, :C],
                                                    scalar1=acol[:, tt:tt + 1])
                        nc.vector.tensor_mul(out=mA[:, st], in0=mA[:, st],
                                             in1=mask_sb[:, st])
                    for th in range(2):
                        po = pS.tile([128, 66], f32, tag="ps")
                        for st in range(2):
                            nc.tensor.matmul(po, mA[:, st, th * 128:(th + 1) * 128],
                                             Va[:, st], start=(st == 0), stop=False)
                        for mi, (m0, mw) in enumerate(MT):
                            nc.tensor.matmul(po, Qp[:mw, mi, th * 128:(th + 1) * 128].bitcast(f32),
                                             S_sb[:mw, mi], start=False, stop=(mi == 2))
                        pc = pS.tile([128, 66], f32, tag="ps")
                        for st in range(2):
                            nc.tensor.matmul(pc, mask_sb[:, st, th * 128:(th + 1) * 128],
                                             Va[:, st], start=(st == 0), stop=(st == 1))
                        rb = sm.tile([128, 1], f32, tag="rb")
                        nc.vector.tensor_scalar_mul(out=rb, in0=po[:, 65:66], scalar1=BETA)
                        tot = sm.tile([128, 66], f32, tag="tot")
                        nc.vector.tensor_scalar_mul(out=tot, in0=pc, scalar1=rb)
                        nc.vector.tensor_add(out=tot, in0=tot, in1=po)
                        den = sm.tile([128, 1], f32, tag="den")
                        nc.vector.tensor_scalar_mul(out=den, in0=tot[:, 65:66],
                                                    scalar1=EPS_C)
                        nc.vector.tensor_add(out=den, in0=den, in1=tot[:, 64:65])
                        nc.vector.reciprocal(out=den, in_=den)
                        ti = ci * 2 + th
                        nc.vector.tensor_scalar_mul(out=o_nat[:, ti, h * 64:(h + 1) * 64],
                                                    in0=tot[:, 0:64], scalar1=den)
                    for mi, (m0, mw) in enumerate(MT):
                        pu = pS.tile([128, 66], f32, tag="ps")
                        for st in range(2):
                            nc.tensor.matmul(pu[:mw], kp[:, st, m0:m0 + mw], Va[:, st],
                                             start=(st == 0), stop=(st == 1))
                        nc.vector.tensor_add(out=S_sb[:mw, mi], in0=S_sb[:mw, mi],
                                             in1=pu[:mw])

            # Wo partial -> AllReduce -> residual + LN2 fused
            oT = big.tile([128, 2, N], bf16, tag="yT")
            for t in range(NT):
                for half in range(2):
                    transpose128(oT[:, half, t * 128:(t + 1) * 128],
                                 o_nat[:, t, half * 128:(half + 1) * 128])
            ccp = cc_in[2 * layer].ap().rearrange("(t p) d -> p t d", p=128)
            for t in range(NT):
                for nh in range(2):
                    ps = pB.tile([128, 512], f32, tag="pb")
                    for kt in range(2):
                        nc.tensor.matmul(ps, oT[:, kt, t * 128:(t + 1) * 128],
                                         wo_sb[:, kt, nh * 512:(nh + 1) * 512],
                                         start=(kt == 0), stop=(kt == 1))
                    ot = ev.tile([128, 512], f32, tag="oev")
                    nc.vector.tensor_copy(out=ot, in_=ps)
                    nc.sync.dma_start(out=ccp[:, t, nh * 512:(nh + 1) * 512], in_=ot)
            nc.gpsimd.collective_compute("AllReduce", Add, ins=[cc_in[2 * layer].ap()],
                                         outs=[cc_out[2 * layer].ap()],
                                         replica_groups=groups)
            cco = cc_out[2 * layer].ap().rearrange("(t p) d -> p t d", p=128)

            w1_sb = wpool.tile([128, DT, FFS], bf16, tag="w1")
            w2_sb = wpool.tile([128, DT, D], bf16, tag="w2")
            b1_sb = wpool.tile([128, DT], f32, tag="b1")
            b2_bc = wpool.tile([128, D], f32, tag="b2bc")
            nc.sync.dma_start(out=w1_sb, in_=w1_in.ap()[layer].rearrange("(k p) c -> p k c", p=128))
            nc.sync.dma_start(out=w2_sb, in_=w2_in.ap()[layer].rearrange("(k p) c -> p k c", p=128))
            nc.sync.dma_start(out=b1_sb, in_=b1_in.ap()[layer].rearrange("(t p) -> p t", p=128))
            bcast_row(b2_bc, b2_in.ap()[layer])

            def load_x_att(t, cco=cco):
                xt = ev.tile([128, D], f32, tag="xtile")
                nc.sync.dma_start(out=xt, in_=xr[:, t])
                at = ev.tile([128, D], f32, tag="arin")
                nc.sync.dma_start(out=at, in_=cco[:, t])
                nc.vector.tensor_add(out=xt, in0=xt, in1=at)
                nc.sync.dma_start(out=xr[:, t], in_=xt)
                return xt

            y2T = ln_transpose_pass(load_x_att)

            ccp = cc_in[2 * layer + 1].ap().rearrange("(t p) d -> p t d", p=128)
            for blk in range(4):
                hT = big.tile([128, DT, 512], bf16, tag="hT")
                for ht in range(DT):
                    ps = pB.tile([128, 512], f32, tag="pb")
                    for kt in range(DT):
                        nc.tensor.matmul(ps, w1_sb[:, kt, ht * 128:(ht + 1) * 128],
                                         y2T[:, kt, blk * 512:(blk + 1) * 512],
                                         start=(kt == 0), stop=(kt == DT - 1))
                    nc.scalar.activation(out=hT[:, ht], in_=ps, func=Gelu,
                                         bias=b1_sb[:, ht:ht + 1])
                for tl in range(4):
                    t = blk * 4 + tl
                    for nh in range(2):
                        ps = pB.tile([128, 512], f32, tag="pb")
                        for kt in range(DT):
                            nc.tensor.matmul(ps, hT[:, kt, tl * 128:(tl + 1) * 128],
                                             w2_sb[:, kt, nh * 512:(nh + 1) * 512],
                                             start=(kt == 0), stop=(kt == DT - 1))
                        ot = ev.tile([128, 512], f32, tag="oev")
                        nc.vector.tensor_add(out=ot, in0=ps,
                                             in1=b2_bc[:, nh * 512:(nh + 1) * 512])
                        nc.sync.dma_start(out=ccp[:, t, nh * 512:(nh + 1) * 512], in_=ot)
            nc.gpsimd.collective_compute("AllReduce", Add, ins=[cc_in[2 * layer + 1].ap()],
                                         outs=[cc_out[2 * layer + 1].ap()],
                                         replica_groups=groups)
            cco2 = cc_out[2 * layer + 1].ap().rearrange("(t p) d -> p t d", p=128)

            if layer < DEPTH - 1:
                for t in range(NT):
                    xt = ev.tile([128, D], f32, tag="xtile")
                    nc.sync.dma_start(out=xt, in_=xr[:, t])
                    at = ev.tile([128, D], f32, tag="arin")
                    nc.sync.dma_start(out=at, in_=cco2[:, t])
                    nc.vector.tensor_add(out=xt, in0=xt, in1=at)
                    nc.sync.dma_start(out=xr[:, t], in_=xt)
            else:
                outp = out_p.ap().rearrange("(t p) d -> p t d", p=128)
                for t in range(NT):
                    xt = ev.tile([128, D], f32, tag="xtile")
                    nc.sync.dma_start(out=xt, in_=xr[:, t])
                    at = ev.tile([128, D], f32, tag="arin")
                    nc.sync.dma_start(out=at, in_=cco2[:, t])
                    nc.vector.tensor_add(out=xt, in0=xt, in1=at)
                    yt = ev.tile([128, D], f32, tag="yln")
                    layernorm_tile(xt, yt)
                    nc.sync.dma_start(out=outp[:, t], in_=yt)

        for p in (pS, pB, pT, hp, ev, smA, sm3, sm, wpool, big, const):
            p.release()

    nc.compile()
    return nc


def _prep_inputs(x, layer_pos_emb, Wq, Wk, Wv, Wo, proj, ln1_g, ln1_b, ln2_g, ln2_b,
                 W1, b1, W2, b2, lnf_g, lnf_b):
    import ml_dtypes
    perm = np.concatenate([np.arange(0, DH, 2), np.arange(1, DH, 2)])
    f = np.float32
    bf = ml_dtypes.bfloat16
    pe = np.asarray(layer_pos_emb, np.float64)[0]
    sinT = np.ascontiguousarray(pe[:, :32].T.astype(f))
    cosT = np.ascontiguousarray(pe[:, 32:].T.astype(f))
    mask = np.ascontiguousarray(
        (np.arange(C)[:, None] <= np.arange(C)[None, :]).astype(f).reshape(2, 128, C))
    maps = []
    for c in range(8):
        b, g = c // TP, c % TP
        heads = range(g * HPC, (g + 1) * HPC)
        colsel = np.concatenate([h * DH + perm for h in heads])
        colsel_v = np.concatenate([h * DH + np.arange(DH) for h in heads])
        wq, wk, wv, bq, bk, bv = [], [], [], [], [], []
        wo, w1, b1h, w2, b2q, pj = [], [], [], [], [], []
        for l in range(DEPTH):
            g1 = np.asarray(ln1_g[l], np.float64)[:, None]
            b1v = np.asarray(ln1_b[l], np.float64)
            Wq_l, Wk_l, Wv_l = (np.asarray(W[l], np.float64) for W in (Wq, Wk, Wv))
            wq.append((g1 * Wq_l)[:, colsel])
            wk.append((g1 * Wk_l)[:, colsel])
            wv.append((g1 * Wv_l)[:, colsel_v])
            bq.append((b1v @ Wq_l)[colsel])
            bk.append((b1v @ Wk_l)[colsel])
            bv.append((b1v @ Wv_l)[colsel_v])
            wo.append(np.asarray(Wo[l], np.float64)[colsel_v, :])
            g2 = np.asarray(ln2_g[l], np.float64)[:, None]
            b2v = np.asarray(ln2_b[l], np.float64)
            W1_l = np.asarray(W1[l], np.float64)
            sl = slice(g * FFS, (g + 1) * FFS)
            w1.append((g2 * W1_l)[:, sl])
            b1h.append((b2v @ W1_l + np.asarray(b1[l], np.float64))[sl])
            w2.append(np.asarray(W2[l], np.float64)[sl, :])
            b2q.append(np.asarray(b2[l], np.float64) / TP)
            pj.append((DN * np.asarray(proj[l], np.float64).T)[perm, :])
        maps.append(dict(
            x=np.ascontiguousarray(np.asarray(x)[b], f),
            cosT=cosT, sinT=sinT, mask=mask,
            wq=np.ascontiguousarray(np.stack(wq).astype(bf)),
            wk=np.ascontiguousarray(np.stack(wk).astype(bf)),
            wv=np.ascontiguousarray(np.stack(wv).astype(bf)),
            bq=np.ascontiguousarray(np.stack(bq), f),
            bk=np.ascontiguousarray(np.stack(bk), f),
            bv=np.ascontiguousarray(np.stack(bv), f),
            wo=np.ascontiguousarray(np.stack(wo).astype(bf)),
            w1=np.ascontiguousarray(np.stack(w1).astype(bf)),
            b1h=np.ascontiguousarray(np.stack(b1h), f),
            w2=np.ascontiguousarray(np.stack(w2).astype(bf)),
            b2q=np.ascontiguousarray(np.stack(b2q), f),
            projT=np.ascontiguousarray(np.stack(pj), f),
        ))
    return maps


def kernel(**inputs):
    global LAST_RESULT
    from concourse.bass_utils import run_bass_kernel_spmd
    if "nc" not in _CACHE:
        _CACHE["nc"] = _build()
    nc = _CACHE["nc"]
    in_maps = _prep_inputs(**inputs)
    res = run_bass_kernel_spmd(nc, in_maps, list(range(8)), trace=TRACE)
    LAST_RESULT = res
    out = np.stack([res.results[0]["out"], res.results[4]["out"]])
    g = np.asarray(inputs["lnf_g"], np.float32)
    b = np.asarray(inputs["lnf_b"], np.float32)
    out = out * g + b
    return np.ascontiguousarray(out.astype(np.float32))



# revision 8
# speedup vs baseline: 1.1447x; 1.1447x over previous
"""Performer (FAVOR+ causal linear attention) encoder on 8 NeuronCores.

Sharding: DP over batch (2 groups of 4 cores) x TP-4 within group
(heads 4-way for attention/QKV/Wo, hidden 4-way for FF). Two AllReduces
per layer over each 4-core group (attention-out partial, FF partial).

GEMMs run bf16xbf16 with fp32 accumulation (weights + transposed
activations cast to bf16); attention feature/scan math stays fp32
(f32r for wide matmuls). Rotary uses a host-side even/odd column
permutation of Wq/Wk so rotate-every-two becomes contiguous 32-row ops.
Attention: chunked prefix scan, C=256. Query features exploit q-side
scale invariance (per-row max/diag cancel in num/denom); key features
are exact, with the k-side eps term restored via a rank-1 correction
(beta * rowsum(q') * mask-matmul cumsum of V).
"""

import numpy as np

B, N, D, H, DH, M, FF, DEPTH = 2, 2048, 1024, 16, 64, 266, 4096, 6
TP = 4
HPC = H // TP            # 4 heads per core
HW = HPC * DH            # 256
FFS = FF // TP           # 1024
C = 256                  # attention chunk
NCH = N // C
NT = N // 128
DT = D // 128
MT = [(0, 128), (128, 128), (256, 10)]
DN = float(DH) ** -0.25
RATIO = float(M) ** -0.5
EPS_F = 1e-4
EPS_C = 1e-6
BETA = RATIO * EPS_F

_CACHE = {}
TRACE = False          # opt-in: capture an NTFF/perfetto trace on next call
LAST_RESULT = None     # BassKernelResults of the most recent run


def _build():
    import concourse.bass as bass
    import concourse.tile as tile
    from concourse import mybir, bacc
    from concourse.masks import make_identity

    f32 = mybir.dt.float32
    bf16 = mybir.dt.bfloat16
    f32r = mybir.dt.float32r

    def r(ap):
        return ap.bitcast(f32r)

    nc = bacc.Bacc("TRN2", target_bir_lowering=False, debug=False, num_devices=8)

    def inp(name, shape, dt=f32):
        return nc.declare_dram_parameter(name, list(shape), dt, isOutput=False)

    x_in = inp("x", [N, D])
    cosT = inp("cosT", [32, N], f32r)
    sinT = inp("sinT", [32, N], f32r)
    mask_in = inp("mask", [2, 128, C])
    wq_in = inp("wq", [DEPTH, D, HW], bf16)
    wk_in = inp("wk", [DEPTH, D, HW], bf16)
    wv_in = inp("wv", [DEPTH, D, HW], bf16)
    bq_in = inp("bq", [DEPTH, HW])
    bk_in = inp("bk", [DEPTH, HW])
    bv_in = inp("bv", [DEPTH, HW])
    wo_in = inp("wo", [DEPTH, HW, D], bf16)
    w1_in = inp("w1", [DEPTH, D, FFS], bf16)
    b1_in = inp("b1h", [DEPTH, FFS])
    w2_in = inp("w2", [DEPTH, FFS, D], bf16)
    b2_in = inp("b2q", [DEPTH, D])
    pj_in = inp("projT", [DEPTH, DH, M], f32r)
    out_p = nc.declare_dram_parameter("out", [N, D], f32, isOutput=True)

    x_res = nc.dram_tensor("x_res", [N, D], f32)
    # per-layer, per-chunk collective buffers (bf16): attn ARs in token
    # quarters, FF ARs in token halves, so each AR overlaps compute.
    ca_in = [[nc.dram_tensor(f"ca_in{l}_{q}", [N // 4, D], f32) for q in range(4)]
             for l in range(DEPTH)]
    ca_out = [[nc.dram_tensor(f"ca_out{l}_{q}", [N // 4, D], f32)
               for q in range(4)] for l in range(DEPTH)]
    cf_in = [[nc.dram_tensor(f"cf_in{l}_{h}", [N // 2, D], f32) for h in range(2)]
             for l in range(DEPTH)]
    cf_out = [[nc.dram_tensor(f"cf_out{l}_{h}", [N // 2, D], f32)
               for h in range(2)] for l in range(DEPTH)]
    mu_dr = [nc.dram_tensor(f"mu{i}", [1, 1], f32) for i in range(DEPTH * HPC)]
    groups = [[0, 1, 2, 3], [4, 5, 6, 7]]

    Exp = mybir.ActivationFunctionType.Exp
    Gelu = mybir.ActivationFunctionType.Gelu_apprx_tanh
    Sqrt = mybir.ActivationFunctionType.Sqrt
    Square = mybir.ActivationFunctionType.Square
    Add = mybir.AluOpType.add
    Sub = mybir.AluOpType.subtract
    Mult = mybir.AluOpType.mult

    def bcast_row(dst, src_ap):
        # dst [128, F] <- broadcast of a [F] dram row across partitions
        nc.sync.dma_start(out=dst, in_=bass.AP(
            tensor=src_ap.tensor, offset=src_ap.offset,
            ap=[[0, 128]] + list(src_ap.ap)))

    with tile.TileContext(nc) as tc:
        const = tc.alloc_tile_pool(name="const", bufs=1)
        big = tc.alloc_tile_pool(name="big", bufs=1)
        wpool = tc.alloc_tile_pool(name="w", bufs=1)
        sm = tc.alloc_tile_pool(name="sm", bufs=2)
        sm3 = tc.alloc_tile_pool(name="sm3", bufs=3)
        smA = tc.alloc_tile_pool(name="smA", bufs=1)
        ev = tc.alloc_tile_pool(name="ev", bufs=2)
        hp = tc.alloc_tile_pool(name="hp", bufs=1)
        pT = tc.alloc_tile_pool(name="pT", bufs=2, space="PSUM")
        pB = tc.alloc_tile_pool(name="pB", bufs=3, space="PSUM")
        pS = tc.alloc_tile_pool(name="pS", bufs=3, space="PSUM")

        ident = const.tile([128, 128], f32)
        make_identity(nc, ident)
        ident_bf = const.tile([128, 128], bf16)
        make_identity(nc, ident_bf)
        cos_sb = const.tile([32, N], f32r)
        sin_sb = const.tile([32, N], f32r)
        nc.sync.dma_start(out=cos_sb, in_=cosT.ap())
        nc.sync.dma_start(out=sin_sb, in_=sinT.ap())
        mask_sb = const.tile([128, 2, C], f32)
        nc.sync.dma_start(out=mask_sb, in_=mask_in.ap().rearrange("s p t -> p s t"))
        ones05 = const.tile([64, 1], f32)
        nc.vector.memset(ones05, 0.5 * DN * DN)
        epsb = const.tile([128, 1], f32)
        nc.vector.memset(epsb, 1e-5)
        lnratio = const.tile([128, 1], f32)
        nc.vector.memset(lnratio, float(np.log(RATIO)))

        nc.sync.dma_start(out=x_res.ap(), in_=x_in.ap())
        xr = x_res.ap().rearrange("(t p) d -> p t d", p=128)

        def layernorm_tile(xt, out_tile):
            st = sm.tile([128, 2, 6], f32, tag="bnst")
            nc.vector.bn_stats(out=st[:, 0], in_=xt[:, 0:512])
            nc.vector.bn_stats(out=st[:, 1], in_=xt[:, 512:1024])
            mv = sm.tile([128, 2], f32, tag="bnmv")
            nc.vector.bn_aggr(out=mv, in_=st)
            rs = sm.tile([128, 1], f32, tag="bnrs")
            nc.scalar.activation(out=rs, in_=mv[:, 1:2], func=Sqrt, bias=epsb)
            nc.vector.reciprocal(out=rs, in_=rs)
            nc.vector.tensor_scalar(out=out_tile, in0=xt, scalar1=mv[:, 0:1],
                                    scalar2=rs, op0=Sub, op1=Mult)

        def transpose128(dst, src):
            ps = pT.tile([128, 128], f32, tag="tps")
            if src.dtype == bf16:
                pp = ps.bitcast(bf16)[: src.shape[-1], : src.shape[0]]
                idn = ident_bf
            else:
                pp = ps[: src.shape[-1], : src.shape[0]]
                idn = ident
            nc.tensor.transpose(pp, src, idn)
            nc.vector.tensor_copy(out=dst, in_=pp)

        def ln_transpose_pass(get_xt, yT_tag="yT"):
            # LN each token tile -> transpose into bf16 [128, DT, N]
            yT = big.tile([128, DT, N], bf16, tag=yT_tag)
            for t in range(NT):
                xt = get_xt(t)
                yt = ev.tile([128, D], f32, tag="yln")
                layernorm_tile(xt, yt)
                for d_ in range(DT):
                    transpose128(yT[:, d_, t * 128:(t + 1) * 128],
                                 yt[:, d_ * 128:(d_ + 1) * 128])
            return yT

        def load_x(t):
            xt = ev.tile([128, D], f32, tag="xtile")
            nc.sync.dma_start(out=xt, in_=xr[:, t])
            return xt

        for layer in range(DEPTH):
            wq_sb = wpool.tile([128, DT, HW], bf16, tag="wq")
            wk_sb = wpool.tile([128, DT, HW], bf16, tag="wk")
            wv_sb = wpool.tile([128, DT, HW], bf16, tag="wv")
            wo_sb = wpool.tile([128, 2, D], bf16, tag="wo")
            pj_sb = wpool.tile([64, M], f32r, tag="pj")
            bq_sb = wpool.tile([64, HPC], f32, tag="bq")
            bk_sb = wpool.tile([64, HPC], f32, tag="bk")
            bv_bc = wpool.tile([128, HW], f32, tag="bvbc")
            nc.sync.dma_start(out=wq_sb, in_=wq_in.ap()[layer].rearrange("(k p) c -> p k c", p=128))
            nc.sync.dma_start(out=wk_sb, in_=wk_in.ap()[layer].rearrange("(k p) c -> p k c", p=128))
            nc.sync.dma_start(out=wv_sb, in_=wv_in.ap()[layer].rearrange("(k p) c -> p k c", p=128))
            nc.sync.dma_start(out=wo_sb, in_=wo_in.ap()[layer].rearrange("(k p) c -> p k c", p=128))
            nc.sync.dma_start(out=pj_sb, in_=pj_in.ap()[layer])
            nc.sync.dma_start(out=bq_sb, in_=bq_in.ap()[layer].rearrange("(h p) -> p h", p=64))
            nc.sync.dma_start(out=bk_sb, in_=bk_in.ap()[layer].rearrange("(h p) -> p h", p=64))
            bcast_row(bv_bc, bv_in.ap()[layer])

            if layer == 0:
                yT = ln_transpose_pass(load_x)
            else:
                def load_x_ff(t, pl=layer - 1):
                    xt = ev.tile([128, D], f32, tag="xtile")
                    nc.sync.dma_start(out=xt, in_=xr[:, t])
                    cco2 = cf_out[pl][t // 8].ap().rearrange("(t p) d -> p t d", p=128)
                    at = ev.tile([128, D], f32, tag="arin")
                    nc.sync.dma_start(out=at, in_=cco2[:, t % 8])
                    nc.vector.tensor_add(out=xt, in0=xt, in1=at)
                    nc.sync.dma_start(out=xr[:, t], in_=xt)
                    return xt
                yT = ln_transpose_pass(load_x_ff)

            # v natural [tok, HW] (bf16, feeds V_aug copies)
            v_nat = big.tile([128, NT, HW], bf16, tag="vnat")
            for t in range(NT):
                ps = pB.tile([128, 512], f32, tag="pb")
                for kt in range(DT):
                    nc.tensor.matmul(ps[:, :HW], yT[:, kt, t * 128:(t + 1) * 128],
                                     wv_sb[:, kt], start=(kt == 0), stop=(kt == DT - 1))
                nc.vector.tensor_add(out=v_nat[:, t], in0=ps[:, :HW], in1=bv_bc)

            o_nat = big.tile([128, NT, HW], bf16, tag="onat")

            for h in range(HPC):
                # qh/kh [64, N] fp32, rotary applied
                qh = hp.tile([64, N], f32r, tag="qh")
                kh = hp.tile([64, N], f32r, tag="kh")
                for (buf, wsb, bsb) in ((qh, wq_sb, bq_sb), (kh, wk_sb, bk_sb)):
                    for blk in range(4):
                        ps = pB.tile([128, 512], f32, tag="pb")
                        for kt in range(DT):
                            nc.tensor.matmul(ps[:64], wsb[:, kt, h * 64:(h + 1) * 64],
                                             yT[:, kt, blk * 512:(blk + 1) * 512],
                                             start=(kt == 0), stop=(kt == DT - 1))
                        nc.vector.tensor_scalar_add(out=buf[:, blk * 512:(blk + 1) * 512],
                                                    in0=ps[:64], scalar1=bsb[:, h:h + 1])
                    # rotary in place: rows [0:32]=E, [32:64]=O (host-permuted).
                    # walrus requires equal SBUF base partitions for DVE
                    # inputs, so O rows bounce through base-0 tiles via DMA.
                    for rb_ in range(2):
                        sl = slice(rb_ * 1024, (rb_ + 1) * 1024)
                        qE = buf[0:32, sl]
                        cs, sn = cos_sb[:, sl], sin_sb[:, sl]
                        qOc = sm3.tile([32, 1024], f32r, tag="rot")
                        t1 = sm3.tile([32, 1024], f32r, tag="rot")
                        t2 = sm3.tile([32, 1024], f32r, tag="rot")
                        nc.sync.dma_start(out=qOc, in_=buf[32:64, sl])
                        nc.vector.tensor_mul(out=t1, in0=qE, in1=cs)
                        nc.vector.tensor_mul(out=t2, in0=qOc, in1=sn)
                        nc.vector.tensor_tensor(out=t1, in0=t1, in1=t2, op=Sub)
                        nc.vector.tensor_mul(out=t2, in0=qOc, in1=cs)
                        nc.vector.tensor_mul(out=qOc, in0=qE, in1=sn)
                        nc.vector.tensor_tensor(out=t2, in0=t2, in1=qOc, op=Add)
                        nc.vector.tensor_copy(out=qE, in_=t1)
                        nc.sync.dma_start(out=buf[32:64, sl], in_=t2)

                # delta_k[t] (col layout) and global max mu of ddk
                dcol = hp.tile([128, NT], f32, tag="dcol")
                mx = hp.tile([128, 12], f32, tag="mxt")
                nc.vector.memset(mx, -1e30)
                for t in range(NT):
                    ksq = sm.tile([64, 128], f32, tag="ksq")
                    nc.scalar.activation(out=ksq, in_=kh[:, t * 128:(t + 1) * 128],
                                         func=Square)
                    psd = pS.tile([128, 66], f32, tag="ps")
                    nc.tensor.matmul(psd[:, 0:1], ksq, ones05, start=True, stop=True)
                    nc.vector.tensor_copy(out=dcol[:, t:t + 1], in_=psd[:, 0:1])
                for mi, (m0, mw) in enumerate(MT):
                    for blk in range(4):
                        ps = pB.tile([128, 512], f32, tag="pb")
                        nc.tensor.matmul(ps[:mw], r(pj_sb[:, m0:m0 + mw]),
                                         r(kh[:, blk * 512:(blk + 1) * 512]),
                                         start=True, stop=True)
                        nc.vector.reduce_max(out=mx[:mw, mi * 4 + blk:mi * 4 + blk + 1],
                                             in_=ps[:mw], axis=mybir.AxisListType.X)
                mxr = sm.tile([128, 1], f32, tag="mxr")
                nc.vector.reduce_max(out=mxr, in_=mx, axis=mybir.AxisListType.X)
                mxT = sm.tile([1, 128], f32, tag="mxT")
                transpose128(mxT, mxr)
                mu = sm.tile([1, 1], f32, tag="mu")
                nc.vector.reduce_max(out=mu, in_=mxT, axis=mybir.AxisListType.X)
                md = mu_dr[layer * HPC + h]
                nc.sync.dma_start(out=md.ap(), in_=mu)
                mub = hp.tile([128, 1], f32, tag="mub")
                nc.sync.dma_start(out=mub, in_=bass.AP(tensor=md, offset=0,
                                                       ap=[[0, 128], [1, 1]]))
                nmu = hp.tile([128, 1], f32, tag="nmu")
                nc.vector.tensor_scalar_mul(out=nmu, in0=mub, scalar1=-1.0)
                acol = hp.tile([128, NT], f32, tag="acol")
                nc.scalar.activation(out=acol, in_=dcol, func=Exp, scale=-1.0,
                                     bias=lnratio)

                S_sb = hp.tile([128, 3, 66], f32, tag="Ssb")
                nc.vector.memset(S_sb, 0.0)
                nc.vector.memset(S_sb[:, :, 65:66], 1.0)

                for ci in range(NCH):
                    c0 = ci * C
                    # per-chunk features
                    EkT = smA.tile([128, 3, C], f32r, tag="EkT")
                    Qp = smA.tile([128, 3, C], f32r, tag="Qp")
                    for mi, (m0, mw) in enumerate(MT):
                        ps = pB.tile([128, 512], f32, tag="pb")
                        nc.tensor.matmul(ps[:mw, :C], r(pj_sb[:, m0:m0 + mw]),
                                         r(kh[:, c0:c0 + C]), start=True, stop=True)
                        nc.scalar.activation(out=EkT[:mw, mi], in_=ps[:mw, :C],
                                             func=Exp, bias=nmu[:mw])
                        ps2 = pB.tile([128, 512], f32, tag="pb")
                        nc.tensor.matmul(ps2[:mw, :C], r(pj_sb[:, m0:m0 + mw]),
                                         r(qh[:, c0:c0 + C]), start=True, stop=True)
                        nc.scalar.activation(out=Qp[:mw, mi], in_=ps2[:mw, :C], func=Exp)
                        nc.vector.tensor_scalar_add(out=Qp[:mw, mi], in0=Qp[:mw, mi],
                                                    scalar1=EPS_F)
                    # k' natural per chunk [tok, M]
                    kp = smA.tile([128, 2, M], f32, tag="kp")
                    for st in range(2):
                        ts0 = c0 + st * 128
                        ps = pB.tile([128, 512], f32, tag="pb")
                        nc.tensor.matmul(ps[:, :M], r(kh[:, ts0:ts0 + 128]), r(pj_sb),
                                         start=True, stop=True)
                        nb = sm.tile([128, 1], f32, tag="nbias")
                        tt = ci * 2 + st
                        nc.vector.tensor_scalar(out=nb, in0=dcol[:, tt:tt + 1],
                                                scalar1=mub, scalar2=-1.0,
                                                op0=Add, op1=Mult)
                        nc.scalar.activation(out=kp[:, st], in_=ps[:, :M], func=Exp,
                                             bias=nb)
                        nc.vector.tensor_scalar(out=kp[:, st], in0=kp[:, st],
                                                scalar1=RATIO, scalar2=RATIO * EPS_F,
                                                op0=Mult, op1=Add)
                    # V_aug
                    Va = smA.tile([128, 2, 66], f32, tag="Va")
                    for st in range(2):
                        nc.vector.tensor_copy(out=Va[:, st, 0:64],
                                              in_=v_nat[:, ci * 2 + st, h * 64:(h + 1) * 64])
                        nc.vector.memset(Va[:, st, 64:65], 1.0)
                        nc.vector.memset(Va[:, st, 65:66], 0.0)
                    # G then masked A
                    mA = smA.tile([128, 2, C], f32, tag="mA")
                    for st in range(2):
                        ts0 = c0 + st * 128
                        pg = pB.tile([128, 512], f32, tag="pb")
                        for mi, (m0, mw) in enumerate(MT):
                            nc.tensor.matmul(pg[:, :C], r(EkT[:mw, mi, st * 128:(st + 1) * 128]),
                                             r(Qp[:mw, mi]), start=(mi == 0), stop=(mi == 2))
                        tt = ci * 2 + st
                        nc.vector.tensor_scalar_mul(out=mA[:, st], in0=pg[:, :C],
                                                    scalar1=acol[:, tt:tt + 1])
                        nc.vector.tensor_mul(out=mA[:, st], in0=mA[:, st],
                                             in1=mask_sb[:, st])
                    for th in range(2):
                        po = pS.tile([128, 66], f32, tag="ps")
                        for st in range(2):
                            nc.tensor.matmul(po, mA[:, st, th * 128:(th + 1) * 128],
                                             Va[:, st], start=(st == 0), stop=False)
                        for mi, (m0, mw) in enumerate(MT):
                            nc.tensor.matmul(po, Qp[:mw, mi, th * 128:(th + 1) * 128].bitcast(f32),
                                             S_sb[:mw, mi], start=False, stop=(mi == 2))
                        pc = pS.tile([128, 66], f32, tag="ps")
                        for st in range(2):
                            nc.tensor.matmul(pc, mask_sb[:, st, th * 128:(th + 1) * 128],
                                             Va[:, st], start=(st == 0), stop=(st == 1))
                        rb = sm.tile([128, 1], f32, tag="rb")
                        nc.vector.tensor_scalar_mul(out=rb, in0=po[:, 65:66], scalar1=BETA)
                        tot = sm.tile([128, 66], f32, tag="tot")
                        nc.vector.tensor_scalar_mul(out=tot, in0=pc, scalar1=rb)
                        nc.vector.tensor_add(out=tot, in0=tot, in1=po)
                        den = sm.tile([128, 1], f32, tag="den")
                        nc.vector.tensor_scalar_mul(out=den, in0=tot[:, 65:66],
                                                    scalar1=EPS_C)
                        nc.vector.tensor_add(out=den, in0=den, in1=tot[:, 64:65])
                        nc.vector.reciprocal(out=den, in_=den)
                        ti = ci * 2 + th
                        nc.vector.tensor_scalar_mul(out=o_nat[:, ti, h * 64:(h + 1) * 64],
                                                    in0=tot[:, 0:64], scalar1=den)
                    for mi, (m0, mw) in enumerate(MT):
                        pu = pS.tile([128, 66], f32, tag="ps")
                        for st in range(2):
                            nc.tensor.matmul(pu[:mw], kp[:, st, m0:m0 + mw], Va[:, st],
                                             start=(st == 0), stop=(st == 1))
                        nc.vector.tensor_add(out=S_sb[:mw, mi], in0=S_sb[:mw, mi],
                                             in1=pu[:mw])

            # FF weights early so the DMA overlaps attention/AR
            w1_sb = wpool.tile([128, DT, FFS], bf16, tag="w1")
            w2_sb = wpool.tile([128, DT, D], bf16, tag="w2")
            b1_sb = wpool.tile([128, DT], f32, tag="b1")
            b2_bc = wpool.tile([128, D], f32, tag="b2bc")
            nc.sync.dma_start(out=w1_sb, in_=w1_in.ap()[layer].rearrange("(k p) c -> p k c", p=128))
            nc.sync.dma_start(out=w2_sb, in_=w2_in.ap()[layer].rearrange("(k p) c -> p k c", p=128))
            nc.sync.dma_start(out=b1_sb, in_=b1_in.ap()[layer].rearrange("(t p) -> p t", p=128))
            bcast_row(b2_bc, b2_in.ap()[layer])

            # Wo partials, one AllReduce per token quarter (bf16 payload) so
            # each AR overlaps the next quarter's GEMMs / LN2+FF.
            for q in range(4):
                ccp = ca_in[layer][q].ap().rearrange("(t p) d -> p t d", p=128)
                for ti in range(4):
                    t = q * 4 + ti
                    oTt = sm.tile([128, 2, 128], bf16, tag="oTt")
                    for half in range(2):
                        transpose128(oTt[:, half],
                                     o_nat[:, t, half * 128:(half + 1) * 128])
                    for nh in range(2):
                        ps = pB.tile([128, 512], f32, tag="pb")
                        for kt in range(2):
                            nc.tensor.matmul(ps, oTt[:, kt],
                                             wo_sb[:, kt, nh * 512:(nh + 1) * 512],
                                             start=(kt == 0), stop=(kt == 1))
                        ot = ev.tile([128, 512], f32, tag="oev")
                        nc.vector.tensor_copy(out=ot, in_=ps)
                        nc.sync.dma_start(out=ccp[:, ti, nh * 512:(nh + 1) * 512], in_=ot)
                nc.gpsimd.collective_compute("AllReduce", Add,
                                             ins=[ca_in[layer][q].ap()],
                                             outs=[ca_out[layer][q].ap()],
                                             replica_groups=groups)

            # LN2 + FF per token half; FF AR per half overlaps the other
            # half's LN2/FF and the next layer's LN1.
            y2T = big.tile([128, DT, N], bf16, tag="yT")
            for h2 in range(2):
                for t in range(h2 * 8, h2 * 8 + 8):
                    xt = ev.tile([128, D], f32, tag="xtile")
                    nc.sync.dma_start(out=xt, in_=xr[:, t])
                    cco = ca_out[layer][t // 4].ap().rearrange("(t p) d -> p t d", p=128)
                    at = ev.tile([128, D], f32, tag="arin")
                    nc.sync.dma_start(out=at, in_=cco[:, t % 4])
                    nc.vector.tensor_add(out=xt, in0=xt, in1=at)
                    nc.sync.dma_start(out=xr[:, t], in_=xt)
                    yt = ev.tile([128, D], f32, tag="yln")
                    layernorm_tile(xt, yt)
                    for d_ in range(DT):
                        transpose128(y2T[:, d_, t * 128:(t + 1) * 128],
                                     yt[:, d_ * 128:(d_ + 1) * 128])
                ccp = cf_in[layer][h2].ap().rearrange("(t p) d -> p t d", p=128)
                for blk in range(h2 * 2, h2 * 2 + 2):
                    hT = big.tile([128, DT, 512], bf16, tag="hT")
                    for ht in range(DT):
                        ps = pB.tile([128, 512], f32, tag="pb")
                        for kt in range(DT):
                            nc.tensor.matmul(ps, w1_sb[:, kt, ht * 128:(ht + 1) * 128],
                                             y2T[:, kt, blk * 512:(blk + 1) * 512],
                                             start=(kt == 0), stop=(kt == DT - 1))
                        nc.scalar.activation(out=hT[:, ht], in_=ps, func=Gelu,
                                             bias=b1_sb[:, ht:ht + 1])
                    for tl in range(4):
                        t = (blk - h2 * 2) * 4 + tl
                        for nh in range(2):
                            ps = pB.tile([128, 512], f32, tag="pb")
                            for kt in range(DT):
                                nc.tensor.matmul(ps, hT[:, kt, tl * 128:(tl + 1) * 128],
                                                 w2_sb[:, kt, nh * 512:(nh + 1) * 512],
                                                 start=(kt == 0), stop=(kt == DT - 1))
                            ot = ev.tile([128, 512], f32, tag="oev")
                            nc.vector.tensor_add(out=ot, in0=ps,
                                                 in1=b2_bc[:, nh * 512:(nh + 1) * 512])
                            nc.sync.dma_start(out=ccp[:, t, nh * 512:(nh + 1) * 512],
                                              in_=ot)
                nc.gpsimd.collective_compute("AllReduce", Add,
                                             ins=[cf_in[layer][h2].ap()],
                                             outs=[cf_out[layer][h2].ap()],
                                             replica_groups=groups)

            if layer == DEPTH - 1:
                outp = out_p.ap().rearrange("(t p) d -> p t d", p=128)
                for t in range(NT):
                    xt = ev.tile([128, D], f32, tag="xtile")
                    nc.sync.dma_start(out=xt, in_=xr[:, t])
                    cco2 = cf_out[layer][t // 8].ap().rearrange("(t p) d -> p t d", p=128)
                    at = ev.tile([128, D], f32, tag="arin")
                    nc.sync.dma_start(out=at, in_=cco2[:, t % 8])
                    nc.vector.tensor_add(out=xt, in0=xt, in1=at)
                    yt = ev.tile([128, D], f32, tag="yln")
                    layernorm_tile(xt, yt)
                    nc.sync.dma_start(out=outp[:, t], in_=yt)

        for p in (pS, pB, pT, hp, ev, smA, sm3, sm, wpool, big, const):
            p.release()

    nc.compile()
    return nc


def _prep_inputs(x, layer_pos_emb, Wq, Wk, Wv, Wo, proj, ln1_g, ln1_b, ln2_g, ln2_b,
                 W1, b1, W2, b2, lnf_g, lnf_b):
    import ml_dtypes
    perm = np.concatenate([np.arange(0, DH, 2), np.arange(1, DH, 2)])
    f = np.float32
    bf = ml_dtypes.bfloat16
    pe = np.asarray(layer_pos_emb, np.float64)[0]
    sinT = np.ascontiguousarray(pe[:, :32].T.astype(f))
    cosT = np.ascontiguousarray(pe[:, 32:].T.astype(f))
    mask = np.ascontiguousarray(
        (np.arange(C)[:, None] <= np.arange(C)[None, :]).astype(f).reshape(2, 128, C))
    maps = []
    for c in range(8):
        b, g = c // TP, c % TP
        heads = range(g * HPC, (g + 1) * HPC)
        colsel = np.concatenate([h * DH + perm for h in heads])
        colsel_v = np.concatenate([h * DH + np.arange(DH) for h in heads])
        wq, wk, wv, bq, bk, bv = [], [], [], [], [], []
        wo, w1, b1h, w2, b2q, pj = [], [], [], [], [], []
        for l in range(DEPTH):
            g1 = np.asarray(ln1_g[l], np.float64)[:, None]
            b1v = np.asarray(ln1_b[l], np.float64)
            Wq_l, Wk_l, Wv_l = (np.asarray(W[l], np.float64) for W in (Wq, Wk, Wv))
            wq.append((g1 * Wq_l)[:, colsel])
            wk.append((g1 * Wk_l)[:, colsel])
            wv.append((g1 * Wv_l)[:, colsel_v])
            bq.append((b1v @ Wq_l)[colsel])
            bk.append((b1v @ Wk_l)[colsel])
            bv.append((b1v @ Wv_l)[colsel_v])
            wo.append(np.asarray(Wo[l], np.float64)[colsel_v, :])
            g2 = np.asarray(ln2_g[l], np.float64)[:, None]
            b2v = np.asarray(ln2_b[l], np.float64)
            W1_l = np.asarray(W1[l], np.float64)
            sl = slice(g * FFS, (g + 1) * FFS)
            w1.append((g2 * W1_l)[:, sl])
            b1h.append((b2v @ W1_l + np.asarray(b1[l], np.float64))[sl])
            w2.append(np.asarray(W2[l], np.float64)[sl, :])
            b2q.append(np.asarray(b2[l], np.float64) / TP)
            pj.append((DN * np.asarray(proj[l], np.float64).T)[perm, :])
        maps.append(dict(
            x=np.ascontiguousarray(np.asarray(x)[b], f),
            cosT=cosT, sinT=sinT, mask=mask,
            wq=np.ascontiguousarray(np.stack(wq).astype(bf)),
            wk=np.ascontiguousarray(np.stack(wk).astype(bf)),
            wv=np.ascontiguousarray(np.stack(wv).astype(bf)),
            bq=np.ascontiguousarray(np.stack(bq), f),
            bk=np.ascontiguousarray(np.stack(bk), f),
            bv=np.ascontiguousarray(np.stack(bv), f),
            wo=np.ascontiguousarray(np.stack(wo).astype(bf)),
            w1=np.ascontiguousarray(np.stack(w1).astype(bf)),
            b1h=np.ascontiguousarray(np.stack(b1h), f),
            w2=np.ascontiguousarray(np.stack(w2).astype(bf)),
            b2q=np.ascontiguousarray(np.stack(b2q), f),
            projT=np.ascontiguousarray(np.stack(pj), f),
        ))
    return maps


def kernel(**inputs):
    global LAST_RESULT
    from concourse.bass_utils import run_bass_kernel_spmd
    if "nc" not in _CACHE:
        _CACHE["nc"] = _build()
    nc = _CACHE["nc"]
    in_maps = _prep_inputs(**inputs)
    res = run_bass_kernel_spmd(nc, in_maps, list(range(8)), trace=TRACE)
    LAST_RESULT = res
    out = np.stack([res.results[0]["out"], res.results[4]["out"]])
    g = np.asarray(inputs["lnf_g"], np.float32)
    b = np.asarray(inputs["lnf_b"], np.float32)
    out = out * g + b
    return np.ascontiguousarray(out.astype(np.float32))

